# revision 34
# baseline (speedup 1.0000x reference)
"""2-layer GCN (gnn_message_passing) on 8 Trainium2 NeuronCores.

Model:  h = relu(A @ (x @ W1) + b1);  out = relu(A @ (h @ W2) + b2)
with A sparse COO (rows=dst, cols=src, vals), 50000 nodes, 800000 edges.

Sharding: nodes (rows) are block-partitioned across the 8 cores; edges
follow their destination row.  The support tables, gathered rows and
selection matrices are bf16 (PSUM accumulation stays fp32).  Each core:
  1. computes its support chunk  s1 = x_chunk @ W1          (PE, bf16)
  2. AllGather -> full support table in HBM (bf16)          (collective)
  3. per 128-row destination tile: dma_gather the source rows (SWDGE,
     4 queues, two tiles merged per gather), source the vals-scaled
     one-hot selection matrix  M[e, d] = vals[e] * (dst_local[e] == d)
     -- the first NS chunks streamed as dense bf16 from DRAM, the rest
     built on DVE (tensor_scalar is_equal+mult against an iota row) --
     and accumulate  psum[f, d] += G[e, f]^T M[e, d]  on the tensor
     engine.  This computes the segment-sum (A @ support)^T.  M is
     identical for both layers: the first KEEP tiles' M stay pinned in
     SBUF and are reused verbatim by layer 2.
  4. relu(+bias) fused on the scalar engine (bias is per-partition
     because the tile is feature-major), then s2 = h @ W2 directly
     (the transposed tile is exactly the lhsT the next matmul needs).
  5. AllGather s2 -> layer-2 table, repeat 3-4, write transposed
     output chunk; the host reassembles / transposes.  The two spmm
     layers interleave at the boundary (layer-2 half-A warmup runs
     during layer-1's half-B tail).

Source indices are split in two halves so they fit dma_gather's int16
index format; per (tile, half) edge lists are padded to a fixed block
count with index-0 / val-0 edges.
"""
import math

import numpy as np

import concourse.bacc as bacc
import concourse.mybir as mybir
import concourse.tile as tile
from concourse import bass_utils, library_config
from concourse.tile_rust import add_dep_helper
import concourse.tile_sem_assignment as _tsa
from concourse.tile_sem_assignment import (
    DMAInst as _DMAInst,
    PROC_NAME_TO_IDX as _PROC_IDX,
)
import concourse.bass_isa as _bass_isa


def _queue_keyed_assign_tick(self, inst):
    """Patched pass-1 lane assignment: SWDGE DMA instructions get their
    DMASW semaphore lane keyed by queue_num (2 lanes per queue) so each
    lane only ever carries one queue's completions - required when mixing
    dma_gather queues with cast dma_start on the default queue."""
    engine = inst.engine
    eng_proc_idx = (
        _tsa.ENGINE_SEQUENCER_TO_IDX if inst.is_sequencer_only()
        else _tsa.ENGINE_TO_IDX
    )[engine]
    if isinstance(inst, _DMAInst) and not isinstance(
        inst, _bass_isa.UserSyncedRemoteDMADescs
    ):
        if engine == mybir.EngineType.Pool:
            q = int(getattr(inst, "queue_num", 0) or 0)
            ctr = getattr(self, "_swdge_q_ctr", None)
            if ctr is None:
                ctr = {}
                self._swdge_q_ctr = ctr
            lane = q * 2 + ctr.get(q, 0) % 2
            ctr[q] = ctr.get(q, 0) + 1
            inst_proc_idx = _PROC_IDX[f"DMASW{lane}"]
        else:
            inst_proc_idx = _PROC_IDX[f"DMAHW{self.next_hw_dma_idx}"]
            self.next_hw_dma_idx = (
                self.next_hw_dma_idx + 1) % _tsa.NUM_HWDGE_SEMS
    elif isinstance(inst, mybir.InstCollectiveCompute):
        inst_proc_idx = _PROC_IDX["Collectives"]
    else:
        inst_proc_idx = eng_proc_idx

    if not inst.is_executable():
        if not isinstance(inst, _tsa.BassTileCriticalSection):
            return
    if isinstance(inst, _bass_isa.InstPseudoReloadLibraryIndex):
        return

    if inst.descendants or isinstance(inst, _tsa._DMA_OR_COLLECTIVE_TYPES):
        inst.bass_scheduled_tick = self.global_clock.advance(inst_proc_idx)
        inst.bass_scheduled_proc = inst_proc_idx
        inst.bass_scheduled_scope = self.scope_name
        self._proc_insts[self.root_scope_name][inst_proc_idx].append(inst)
        if getattr(inst, "gen_mode", 0) == 1 and inst_proc_idx != eng_proc_idx:
            eng_tick = self.global_clock.advance(eng_proc_idx)
            self.tc.prep_eng_ticks[inst.name] = (eng_proc_idx, eng_tick)
            self._prep_eng_names[self.root_scope_name].append(inst.name)


_tsa.TileClockTick._assign_tick = _queue_keyed_assign_tick

P = 128
N_CORES = 8

# full-size problem geometry (hardcoded per the task spec)
N_NODES = 50000
F_IN = 256
H = 128


class Cfg:
    def __init__(self, n_nodes, f_in, h, tiles_per_core, kh):
        self.n_nodes = n_nodes
        self.f_in = f_in
        self.h = h
        self.tiles_per_core = tiles_per_core
        self.rows_per_core = tiles_per_core * P
        self.n_pad = self.rows_per_core * N_CORES
        # tile-aligned split of each core's chunk into half a / half b;
        # the gather tables are [core-major] per half so indices fit int16
        self.ta = (tiles_per_core + 1) // 2
        self.tb = tiles_per_core - self.ta
        self.rows_a = self.ta * P
        self.rows_b = self.tb * P
        self.n_a = self.rows_a * N_CORES
        self.n_b = self.rows_b * N_CORES
        assert self.n_a <= 32768 and self.n_b <= 32768
        self.kh = kh                  # gather blocks per (tile, half)
        self.nidx = kh * P            # rows per dma_gather
        self.icols = self.nidx // 16  # int16 idx columns per gather
        self.kb = 2 * kh              # edge blocks per tile
        self.ns = min(4, kh)          # m chunks streamed (rest DVE-built)
        assert f_in % P == 0 and h == P


def build_program(cfg: Cfg):
    nc = bacc.Bacc("TRN2", target_bir_lowering=False, debug=False,
                   num_devices=N_CORES, num_swdge_queues=4,
                   dynamic_dma_scratch_size=0x8000)
    T = cfg.tiles_per_core
    KH, KB, ICOLS = cfg.kh, cfg.kb, cfg.icols
    KC = cfg.f_in // P  # k-chunks for the first dense matmul
    LAG = 8             # how many tiles the half-B pipeline trails half-A
    GM = 2              # tiles per merged gather
    NS = cfg.ns         # selection-matrix chunks streamed from DRAM (rest
    #                     built on DVE) -- balances DMA bytes vs DVE time

    dt = mybir.dt
    featT = nc.dram_tensor("featT", [P, KC * cfg.rows_per_core], dt.bfloat16,
                           kind="ExternalInput")
    w1 = nc.dram_tensor("w1", [P, KC * H], dt.bfloat16, kind="ExternalInput")
    w2 = nc.dram_tensor("w2", [H, H], dt.bfloat16, kind="ExternalInput")
    b1 = nc.dram_tensor("b1", [H, 1], dt.float32, kind="ExternalInput")
    b2 = nc.dram_tensor("b2", [H, 1], dt.float32, kind="ExternalInput")
    idx16 = nc.dram_tensor("idx16", [P, T * 2 * ICOLS], dt.int16,
                           kind="ExternalInput")
    m01 = nc.dram_tensor("m01", [P, T * 2 * NS * P], dt.bfloat16,
                         kind="ExternalInput")
    dstv = nc.dram_tensor("dstv", [P, T * KB], dt.float32,
                          kind="ExternalInput")
    valv = nc.dram_tensor("valv", [P, T * KB], dt.float32,
                          kind="ExternalInput")
    outT = nc.dram_tensor("outT", [H, cfg.rows_per_core], dt.float32,
                          kind="ExternalOutput")

    sup1_loc = nc.dram_tensor("sup1_loc", [cfg.rows_per_core, H], dt.bfloat16,
                              kind="Internal")
    sup1a = nc.dram_tensor("sup1a", [cfg.n_a, H], dt.bfloat16,
                           kind="Internal", addr_space="Shared")
    sup1b = nc.dram_tensor("sup1b", [cfg.n_b, H], dt.bfloat16,
                           kind="Internal", addr_space="Shared")
    sup2_loc = nc.dram_tensor("sup2_loc", [cfg.rows_per_core, H], dt.bfloat16,
                              kind="Internal")
    sup2a = nc.dram_tensor("sup2a", [cfg.n_a, H], dt.bfloat16,
                           kind="Internal", addr_space="Shared")
    sup2b = nc.dram_tensor("sup2b", [cfg.n_b, H], dt.bfloat16,
                           kind="Internal", addr_space="Shared")

    groups = [list(range(N_CORES))]

    with tile.TileContext(nc) as tc:
        with (
            tc.tile_pool(name="meta", bufs=1) as meta,
            tc.tile_pool(name="sA", bufs=17) as sApool,
            tc.tile_pool(name="spool", bufs=6) as spool,
            tc.tile_pool(name="ps1", bufs=2, space="PSUM") as ps1p,
            tc.tile_pool(name="psA", bufs=2, space="PSUM") as psAp,
            tc.tile_pool(name="psB", bufs=4, space="PSUM") as psBp,
        ):
            lib = nc.gpsimd.load_library(library_config.mlp)

            # ---- constant / metadata loads ----
            idx_sb = meta.tile([P, T * 2 * ICOLS], dt.int16)
            nc.sync.dma_start(idx_sb[:], idx16[:])
            w1_sb = meta.tile([P, KC * H], dt.bfloat16)
            nc.sync.dma_start(w1_sb[:], w1[:])
            w2_sb = meta.tile([H, H], dt.bfloat16)
            nc.sync.dma_start(w2_sb[:], w2[:])
            b1_sb = meta.tile([H, 1], dt.float32)
            nc.sync.dma_start(b1_sb[:], b1[:])
            b2_sb = meta.tile([H, 1], dt.float32)
            nc.sync.dma_start(b2_sb[:], b2[:])
            dstv_sb = meta.tile([P, T * KB], dt.float32)
            nc.sync.dma_start(dstv_sb[:], dstv[:])
            valv_sb = meta.tile([P, T * KB], dt.float32)
            nc.sync.dma_start(valv_sb[:], valv[:])
            # iota row 0..127 along free dim, same in every partition;
            # generated as int32 then converted to bf16 (exact for 0..127)
            iota32 = meta.tile([P, P], dt.int32)
            nc.gpsimd.iota(iota32[:], pattern=[[1, P]], channel_multiplier=0)
            iota_bf = meta.tile([P, P], dt.bfloat16)
            nc.scalar.copy(iota_bf[:], iota32[:])

            # featT streamed in 4 slabs of tiles (pool freed after phase B)
            featB = tc.tile_pool(name="featB", bufs=1)
            featpool = featB.__enter__()
            featT_sb = featpool.tile([P, KC * cfg.rows_per_core], dt.bfloat16)
            _spmm_pools = None
            nslab = 4
            per = (T + nslab - 1) // nslab
            for sl in range(nslab):
                a = sl * per * P
                b = min(T, (sl + 1) * per) * P
                if a >= b:
                    continue
                for kc in range(KC):
                    off = kc * cfg.rows_per_core
                    eng = (nc.sync, nc.scalar)[(sl * KC + kc) % 2]
                    eng.dma_start(featT_sb[:, off + a:off + b],
                                  featT[:, off + a:off + b])

            def ag(ins_ap, out_t):
                nc.gpsimd.collective_compute(
                    "AllGather", mybir.AluOpType.bypass,
                    replica_groups=groups, ins=[ins_ap], outs=[out_t[:]])

            # ---- phase B: local support1 chunk; AG each half when ready --
            for t in range(T):
                ps = ps1p.tile([P, H], dt.float32, tag="ps1")
                for kc in range(KC):
                    nc.tensor.matmul(
                        ps[:],
                        lhsT=featT_sb[:, kc * cfg.rows_per_core + t * P:
                                      kc * cfg.rows_per_core + (t + 1) * P],
                        rhs=w1_sb[:, kc * H:(kc + 1) * H],
                        start=(kc == 0), stop=(kc == KC - 1))
                s1_sb = spool.tile([P, H], dt.bfloat16, tag="s1")
                nc.scalar.copy(s1_sb[:], ps[:])
                nc.sync.dma_start(sup1_loc[t * P:(t + 1) * P, :], s1_sb[:])
                if t == min(cfg.ta, T - 1):
                    ag(sup1_loc[0:cfg.rows_a, :], sup1a)
            ag(sup1_loc[cfg.rows_a:, :], sup1b)
            featB.__exit__(None, None, None)
            _cms = [tc.tile_pool(name="gA", bufs=6),
                    tc.tile_pool(name="gB", bufs=6),
                    tc.tile_pool(name="mA", bufs=10),
                    tc.tile_pool(name="mB", bufs=10),
                    tc.tile_pool(name="mK", bufs=1)]
            gApool, gBpool, mApool, mBpool, mKpool = \
                [c.__enter__() for c in _cms]
            KEEP = 11   # tiles whose m stays in SBUF across both layers
            m_kept = {}

            def emit_gather(g, t, half, tab, ntiles):
                """one gather covering `ntiles` consecutive tiles' `half`
                blocks (idx16 is laid out group-blocked, see prep_inputs)"""
                k = ntiles * KH
                out_ap = g[:].rearrange("p (k h) -> p k h", k=k)
                c0 = (t * 2 + half * ntiles) * ICOLS
                gi = nc.gpsimd.dma_gather(
                    out_ap=out_ap, in_ap=tab[:],
                    idxs_ap=idx_sb[:, c0:c0 + ntiles * ICOLS],
                    num_idxs=ntiles * cfg.nidx,
                    num_idxs_reg=ntiles * cfg.nidx,
                    elem_size=H, single_packet=False,
                    queue_num=((t // GM) * 2 + half) % 4)
                add_dep_helper(lib.ins, gi.ins, sync=False,
                               reason="lib before gather")

            class SpmmPhase:
                """software-pipelined spmm: half-A work leads by LAG tiles.

                Per tile: gather half-A rows (pair-merged gathers), source
                the vals-scaled selection matrix (first NS chunks streamed
                from DRAM, the rest built on DVE from (dst, val) scalars),
                matmul-accumulate, stash the partial (PSUM -> SBUF).  LAG
                tiles later the half-B gather + chunks run (their table's
                AllGather has completed by then), the partials combine,
                relu(+bias), and out_cb consumes the transposed tile.
                """

                def __init__(self, tab_a, tab_b, bias_sb, out_cb, layer):
                    self.tab = (tab_a, tab_b)
                    self.bias_sb = bias_sb
                    self.out_cb = out_cb
                    self.layer = layer
                    self.sA = {}
                    self.pend = ({}, {})

                def _gather(self, t, half):
                    pool = (gApool, gBpool)[half]
                    pend = self.pend[half]
                    nt = min(GM, T - t)
                    g = pool.tile([P, nt * KH * P], dt.bfloat16,
                                  tag="gab"[half])
                    emit_gather(g, t, half, self.tab[half], nt)
                    for i in range(nt):
                        pend[t + i] = (g, i * KH * P)

                def _m_tile(self, t, half):
                    if t < KEEP:
                        # m is layer-independent: build once (layer 0) in a
                        # pinned buffer, reuse verbatim in layer 1
                        if self.layer == 1:
                            return m_kept[(t, half)]
                        m = mKpool.tile([P, KH * P], dt.bfloat16,
                                        tag=f"mk{t}_{half}")
                        m_kept[(t, half)] = m
                    else:
                        m = (mApool, mBpool)[half].tile(
                            [P, KH * P], dt.bfloat16, tag="mab"[half])
                    eng = (nc.sync, nc.scalar)[half]
                    eng.dma_start(m[:, :NS * P],
                                  m01[:, (t * 2 + half) * NS * P:
                                      (t * 2 + half + 1) * NS * P])
                    for j in range(NS, KH):
                        c = t * KB + half * KH + j
                        nc.vector.tensor_scalar(
                            out=m[:, j * P:(j + 1) * P], in0=iota_bf[:],
                            scalar1=dstv_sb[:, c:c + 1],
                            scalar2=valv_sb[:, c:c + 1],
                            op0=mybir.AluOpType.is_equal,
                            op1=mybir.AluOpType.mult)
                    return m

                def _chunks(self, t, half, ps):
                    g, off = self.pend[half].pop(t)
                    m = self._m_tile(t, half)
                    for j in range(KH):
                        nc.tensor.matmul(
                            ps[:], lhsT=g[:, off + j * P:off + (j + 1) * P],
                            rhs=m[:, j * P:(j + 1) * P],
                            start=(j == 0), stop=(j == KH - 1))

                def a_work(self, t):
                    if t % GM == 0:
                        self._gather(t, 0)
                    ps = psAp.tile([P, P], dt.float32, tag="cha")
                    self._chunks(t, 0, ps)
                    sa = sApool.tile([P, P], dt.float32, tag="sa")
                    nc.scalar.copy(sa[:], ps[:])
                    self.sA[t] = sa

                def b_work(self, t):
                    if t % GM == 0:
                        self._gather(t, 1)
                    ps = psBp.tile([P, P], dt.float32, tag="chb")
                    self._chunks(t, 1, ps)
                    # combine halves + relu(+bias)
                    sa = self.sA.pop(t)
                    hc = spool.tile([P, P], dt.float32, tag="hc")
                    nc.vector.tensor_tensor(out=hc[:], in0=ps[:], in1=sa[:],
                                            op=mybir.AluOpType.add)
                    hT = spool.tile([P, P],
                                    dt.bfloat16 if self.layer == 0
                                    else dt.float32, tag=f"hT{self.layer}")
                    nc.scalar.activation(hT[:], hc[:],
                                         mybir.ActivationFunctionType.Relu,
                                         bias=self.bias_sb[:], scale=1.0)
                    self.out_cb(t, hT)

            def l1_out(t, hT):
                ps2 = psBp.tile([P, H], dt.float32, tag="chb")
                nc.tensor.matmul(ps2[:], lhsT=hT[:], rhs=w2_sb[:],
                                 start=True, stop=True)
                s2_sb = spool.tile([P, H], dt.bfloat16, tag="s2")
                nc.scalar.copy(s2_sb[:], ps2[:])
                nc.sync.dma_start(sup2_loc[t * P:(t + 1) * P, :], s2_sb[:])

            def l2_out(t, oT):
                nc.sync.dma_start(outT[:, t * P:(t + 1) * P], oT[:])

            # ---- phases D & F: the two spmm layers, interleaved at the
            # boundary: phase F's half-A warmup runs during phase D's
            # half-B tail so the gather queues never drain.
            dph = SpmmPhase(sup1a, sup1b, b1_sb, l1_out, 0)
            fph = SpmmPhase(sup2a, sup2b, b2_sb, l2_out, 1)
            lag = min(LAG, T - 1)
            tmid = min(cfg.ta + lag + 2, T - 1)
            for t in range(T):
                dph.a_work(t)
                if t >= lag:
                    dph.b_work(t - lag)
                if t == tmid:
                    ag(sup2_loc[0:cfg.rows_a, :], sup2a)
            fh = 0
            for i, t in enumerate(range(T - lag, T)):
                dph.b_work(t)
                if i >= 2:
                    fph.a_work(fh)
                    fh += 1
            ag(sup2_loc[cfg.rows_a:, :], sup2b)
            lag_f = min(LAG + 4, T - 1)
            for t in range(fh, T):
                fph.a_work(t)
                if t >= lag_f:
                    fph.b_work(t - lag_f)
            for t in range(T - lag_f, T):
                fph.b_work(t)
            for c in reversed(_cms):
                c.__exit__(None, None, None)

    nc.compile()
    return nc


def prep_inputs(features, adj_rows, adj_cols, adj_vals, W1, b1, W2, b2,
                cfg: Cfg):
    """Host-side sharding: per-core featT chunks and per-tile edge lists."""
    rows = np.asarray(adj_rows, dtype=np.int64)
    cols = np.asarray(adj_cols, dtype=np.int64)
    vals = np.asarray(adj_vals, dtype=np.float32)
    feats = np.asarray(features, dtype=np.float32)
    n, f_in = feats.shape
    T = cfg.tiles_per_core
    KH, KB, ICOLS, NIDX = cfg.kh, cfg.kb, cfg.icols, cfg.nidx
    n_tiles = T * N_CORES

    # pad features to n_pad rows
    featsp = np.zeros((cfg.n_pad, f_in), np.float32)
    featsp[:n] = feats

    # sort edges by destination tile, then split by source half
    # (half = tile-aligned half of the source's core chunk)
    tile_of = rows // P
    order = np.argsort(tile_of, kind="stable")
    rows_s, cols_s, vals_s = rows[order], cols[order], vals[order]
    src_core = cols_s // cfg.rows_per_core
    src_r = cols_s % cfg.rows_per_core
    half_s = (src_r >= cfg.rows_a).astype(np.int64)
    # position within the per-half core-major gather table
    pos_s = np.where(half_s == 0,
                     src_core * cfg.rows_a + src_r,
                     src_core * cfg.rows_b + (src_r - cfg.rows_a))
    counts = np.bincount(tile_of[order] * 2 + half_s, minlength=n_tiles * 2)
    kmax = counts.max()
    assert kmax <= NIDX, f"tile/half edge count {kmax} exceeds {NIDX}"

    # order edges by (tile, half) for slotting
    order2 = np.argsort(tile_of[order] * 2 + half_s, kind="stable")
    rows_s, cols_s, vals_s = rows_s[order2], cols_s[order2], vals_s[order2]
    pos_s = pos_s[order2]

    # slot positions within each (tile, half) group
    grp = np.repeat(np.arange(n_tiles * 2), counts)
    starts = np.concatenate([[0], np.cumsum(counts)[:-1]])
    slot = np.arange(len(rows_s)) - starts[grp]

    # dense [n_tiles, 2, NIDX] arrays, padded with idx=0 / val=0
    idx_d = np.zeros((n_tiles, 2, NIDX), np.int64)
    dst_d = np.zeros((n_tiles, 2, NIDX), np.int16)
    val_d = np.zeros((n_tiles, 2, NIDX), np.float32)
    tile_idx = grp // 2
    half_idx = grp % 2
    idx_d[tile_idx, half_idx, slot] = pos_s
    dst_d[tile_idx, half_idx, slot] = (rows_s % P).astype(np.int16)
    val_d[tile_idx, half_idx, slot] = vals_s

    import ml_dtypes
    bf16 = ml_dtypes.bfloat16

    # group-blocked gather order: per group of GM tiles starting at t0,
    # [a of t0..t0+s-1, b of t0..t0+s-1]
    GM = 2
    blk_t, blk_h = [], []
    t = 0
    while t < T:
        s = min(GM, T - t)
        blk_t += list(range(t, t + s)) * 2
        blk_h += [0] * s + [1] * s
        t += s
    blk_t = np.array(blk_t)
    blk_h = np.array(blk_h)

    in_maps = []
    for c in range(N_CORES):
        t0 = c * T
        chunk = featsp[c * cfg.rows_per_core:(c + 1) * cfg.rows_per_core]
        # [rows, f_in] -> [P, KC*rows]: chunk kc on cols [kc*rows, (kc+1)*rows)
        kc_n = cfg.f_in // P
        featT = np.ascontiguousarray(
            chunk.T.reshape(kc_n, P, cfg.rows_per_core).transpose(1, 0, 2)
            .reshape(P, kc_n * cfg.rows_per_core)).astype(bf16)
        # idx16: [P, T*2*ICOLS], pair-grouped blocks, 16-wrapped and
        # replicated over 8 groups
        idx_c = idx_d[t0:t0 + T][blk_t, blk_h]          # [T*2, NIDX]
        idx_c = idx_c.reshape(T * 2, ICOLS, 16).astype(np.int16)
        idx16 = np.tile(idx_c.transpose(0, 2, 1), (1, 8, 1))  # [T*2, 128, IC]
        idx16 = idx16.transpose(1, 0, 2).reshape(P, T * 2 * ICOLS)
        # vals-scaled one-hot selection matrices: the first NS chunks per
        # (tile, half) are streamed as dense bf16 (m01); all chunks' (dst,
        # val) scalars ship as fp32 for the on-chip DVE build of the rest
        NS = cfg.ns
        dst_c = dst_d[t0:t0 + T].reshape(T, KB, P)          # [t, jj, p]
        val_c = val_d[t0:t0 + T].reshape(T, KB, P)          # [t, jj, p]
        dstv = np.ascontiguousarray(
            dst_c.transpose(2, 0, 1).reshape(P, T * KB).astype(np.float32))
        valv = np.ascontiguousarray(
            val_c.transpose(2, 0, 1).reshape(P, T * KB).astype(np.float32))
        # streamed chunks: jj in [0, NS) and [KH, KH+NS) per tile
        sjj = np.concatenate([np.arange(NS), KH + np.arange(NS)])
        dst_s = dst_c[:, sjj]                               # [t, 2*NS, p]
        val_s = val_c[:, sjj]
        m01 = np.zeros((T, 2 * NS, P, P), bf16)             # [t, sj, p, d]
        ti, ji, pi = np.indices(dst_s.shape, sparse=True)
        m01[ti, ji, pi, dst_s] = val_s.astype(bf16)
        m01 = np.ascontiguousarray(
            m01.transpose(2, 0, 1, 3).reshape(P, T * 2 * NS * P))
        in_maps.append({
            "featT": featT,
            "w1": np.ascontiguousarray(
                np.asarray(W1, np.float32).reshape(kc_n, P, H)
                .transpose(1, 0, 2).reshape(P, kc_n * H)).astype(bf16),
            "w2": np.asarray(W2, np.float32).astype(bf16),
            "b1": np.asarray(b1, np.float32).reshape(H, 1),
            "b2": np.asarray(b2, np.float32).reshape(H, 1),
            "idx16": np.ascontiguousarray(idx16),
            "m01": m01,
            "dstv": dstv,
            "valv": valv,
        })
    return in_maps


_CACHED = {}


def run(features, adj_rows, adj_cols, adj_vals, W1, b1, W2, b2, cfg,
        trace=False):
    key = (cfg.n_pad, cfg.kh, cfg.tiles_per_core)
    if key not in _CACHED:
        _CACHED[key] = build_program(cfg)
    nc = _CACHED[key]
    in_maps = prep_inputs(features, adj_rows, adj_cols, adj_vals,
                          W1, b1, W2, b2, cfg)
    res = bass_utils.run_bass_kernel_spmd(nc, in_maps, list(range(N_CORES)),
                                          trace=trace)
    outs = [res.results[c]["outT"] for c in range(N_CORES)]
    full = np.concatenate([o.T for o in outs], axis=0)  # [n_pad, H]
    return full[:features.shape[0]].astype(np.float32), res


def kernel(features, adj_rows, adj_cols, adj_vals, W1, b1, W2, b2):
    rows = np.asarray(adj_rows, dtype=np.int64)
    cols = np.asarray(adj_cols, dtype=np.int64)
    # KH sized from the actual data (block-rounded max per (tile, half))
    cfg0 = Cfg(N_NODES, F_IN, H, tiles_per_core=49, kh=2)
    tile_of = rows // P
    half_s = ((cols % cfg0.rows_per_core) >= cfg0.rows_a).astype(np.int64)
    counts = np.bincount(tile_of * 2 + half_s,
                         minlength=cfg0.tiles_per_core * N_CORES * 2)
    kh = max(2, math.ceil(counts.max() / P))
    cfg = Cfg(N_NODES, F_IN, H, tiles_per_core=49, kh=kh)
    out, _ = run(features, adj_rows, adj_cols, adj_vals, W1, b1, W2, b2, cfg)
    return out



# revision 35
# speedup vs baseline: 1.0020x; 1.0020x over previous
"""2-layer GCN (gnn_message_passing) on 8 Trainium2 NeuronCores.

Model:  h = relu(A @ (x @ W1) + b1);  out = relu(A @ (h @ W2) + b2)
with A sparse COO (rows=dst, cols=src, vals), 50000 nodes, 800000 edges.

Sharding: nodes (rows) are block-partitioned across the 8 cores; edges
follow their destination row.  The support tables, gathered rows and
selection matrices are bf16 (PSUM accumulation stays fp32).  Each core:
  1. computes its support chunk  s1 = x_chunk @ W1          (PE, bf16)
  2. AllGather -> full support table in HBM (bf16)          (collective)
  3. per 128-row destination tile: dma_gather the source rows (SWDGE,
     4 queues, two tiles merged per gather), source the vals-scaled
     one-hot selection matrix  M[e, d] = vals[e] * (dst_local[e] == d)
     -- the first NS chunks streamed as dense bf16 from DRAM, the rest
     built on DVE (tensor_scalar is_equal+mult against an iota row) --
     and accumulate  psum[f, d] += G[e, f]^T M[e, d]  on the tensor
     engine.  This computes the segment-sum (A @ support)^T.  M is
     identical for both layers: the first KEEP tiles' M stay pinned in
     SBUF and are reused verbatim by layer 2.
  4. relu(+bias) fused on the scalar engine (bias is per-partition
     because the tile is feature-major), then s2 = h @ W2 directly
     (the transposed tile is exactly the lhsT the next matmul needs).
  5. AllGather s2 -> layer-2 table, repeat 3-4, write transposed
     output chunk; the host reassembles / transposes.  The two spmm
     layers interleave at the boundary (layer-2 half-A warmup runs
     during layer-1's half-B tail).

Source indices are split in two halves so they fit dma_gather's int16
index format; per (tile, half) edge lists are padded to a fixed block
count with index-0 / val-0 edges.
"""
import math

import numpy as np

import concourse.bacc as bacc
import concourse.mybir as mybir
import concourse.tile as tile
from concourse import bass_utils, library_config
from concourse.tile_rust import add_dep_helper
import concourse.tile_sem_assignment as _tsa
from concourse.tile_sem_assignment import (
    DMAInst as _DMAInst,
    PROC_NAME_TO_IDX as _PROC_IDX,
)
import concourse.bass_isa as _bass_isa


def _queue_keyed_assign_tick(self, inst):
    """Patched pass-1 lane assignment: SWDGE DMA instructions get their
    DMASW semaphore lane keyed by queue_num (2 lanes per queue) so each
    lane only ever carries one queue's completions - required when mixing
    dma_gather queues with cast dma_start on the default queue."""
    engine = inst.engine
    eng_proc_idx = (
        _tsa.ENGINE_SEQUENCER_TO_IDX if inst.is_sequencer_only()
        else _tsa.ENGINE_TO_IDX
    )[engine]
    if isinstance(inst, _DMAInst) and not isinstance(
        inst, _bass_isa.UserSyncedRemoteDMADescs
    ):
        if engine == mybir.EngineType.Pool:
            q = int(getattr(inst, "queue_num", 0) or 0)
            ctr = getattr(self, "_swdge_q_ctr", None)
            if ctr is None:
                ctr = {}
                self._swdge_q_ctr = ctr
            lane = q * 2 + ctr.get(q, 0) % 2
            ctr[q] = ctr.get(q, 0) + 1
            inst_proc_idx = _PROC_IDX[f"DMASW{lane}"]
        else:
            inst_proc_idx = _PROC_IDX[f"DMAHW{self.next_hw_dma_idx}"]
            self.next_hw_dma_idx = (
                self.next_hw_dma_idx + 1) % _tsa.NUM_HWDGE_SEMS
    elif isinstance(inst, mybir.InstCollectiveCompute):
        inst_proc_idx = _PROC_IDX["Collectives"]
    else:
        inst_proc_idx = eng_proc_idx

    if not inst.is_executable():
        if not isinstance(inst, _tsa.BassTileCriticalSection):
            return
    if isinstance(inst, _bass_isa.InstPseudoReloadLibraryIndex):
        return

    if inst.descendants or isinstance(inst, _tsa._DMA_OR_COLLECTIVE_TYPES):
        inst.bass_scheduled_tick = self.global_clock.advance(inst_proc_idx)
        inst.bass_scheduled_proc = inst_proc_idx
        inst.bass_scheduled_scope = self.scope_name
        self._proc_insts[self.root_scope_name][inst_proc_idx].append(inst)
        if getattr(inst, "gen_mode", 0) == 1 and inst_proc_idx != eng_proc_idx:
            eng_tick = self.global_clock.advance(eng_proc_idx)
            self.tc.prep_eng_ticks[inst.name] = (eng_proc_idx, eng_tick)
            self._prep_eng_names[self.root_scope_name].append(inst.name)


_tsa.TileClockTick._assign_tick = _queue_keyed_assign_tick

P = 128
N_CORES = 8

# full-size problem geometry (hardcoded per the task spec)
N_NODES = 50000
F_IN = 256
H = 128


class Cfg:
    def __init__(self, n_nodes, f_in, h, tiles_per_core, kh):
        self.n_nodes = n_nodes
        self.f_in = f_in
        self.h = h
        self.tiles_per_core = tiles_per_core
        self.rows_per_core = tiles_per_core * P
        self.n_pad = self.rows_per_core * N_CORES
        # tile-aligned split of each core's chunk into half a / half b;
        # the gather tables are [core-major] per half so indices fit int16
        self.ta = (tiles_per_core + 1) // 2
        self.tb = tiles_per_core - self.ta
        self.rows_a = self.ta * P
        self.rows_b = self.tb * P
        self.n_a = self.rows_a * N_CORES
        self.n_b = self.rows_b * N_CORES
        assert self.n_a <= 32768 and self.n_b <= 32768
        self.kh = kh                  # gather blocks per (tile, half)
        self.nidx = kh * P            # rows per dma_gather
        self.icols = self.nidx // 16  # int16 idx columns per gather
        self.kb = 2 * kh              # edge blocks per tile
        self.ns = min(4, kh)          # m chunks streamed (rest DVE-built)
        assert f_in % P == 0 and h == P


def build_program(cfg: Cfg):
    nc = bacc.Bacc("TRN2", target_bir_lowering=False, debug=False,
                   num_devices=N_CORES, num_swdge_queues=4,
                   dynamic_dma_scratch_size=0x8000)
    T = cfg.tiles_per_core
    KH, KB, ICOLS = cfg.kh, cfg.kb, cfg.icols
    KC = cfg.f_in // P  # k-chunks for the first dense matmul
    LAG = 8             # how many tiles the half-B pipeline trails half-A
    GM = 2              # tiles per merged gather
    NS = cfg.ns         # selection-matrix chunks streamed from DRAM (rest
    #                     built on DVE) -- balances DMA bytes vs DVE time

    dt = mybir.dt
    featT = nc.dram_tensor("featT", [P, KC * cfg.rows_per_core], dt.bfloat16,
                           kind="ExternalInput")
    w1 = nc.dram_tensor("w1", [P, KC * H], dt.bfloat16, kind="ExternalInput")
    w2 = nc.dram_tensor("w2", [H, H], dt.bfloat16, kind="ExternalInput")
    b1 = nc.dram_tensor("b1", [H, 1], dt.float32, kind="ExternalInput")
    b2 = nc.dram_tensor("b2", [H, 1], dt.float32, kind="ExternalInput")
    idx16 = nc.dram_tensor("idx16", [P, T * 2 * ICOLS], dt.int16,
                           kind="ExternalInput")
    m01 = nc.dram_tensor("m01", [P, T * 2 * NS * P], dt.bfloat16,
                         kind="ExternalInput")
    dstv = nc.dram_tensor("dstv", [P, T * KB], dt.float32,
                          kind="ExternalInput")
    valv = nc.dram_tensor("valv", [P, T * KB], dt.float32,
                          kind="ExternalInput")
    outT = nc.dram_tensor("outT", [H, cfg.rows_per_core], dt.float32,
                          kind="ExternalOutput")

    sup1_loc = nc.dram_tensor("sup1_loc", [cfg.rows_per_core, H], dt.bfloat16,
                              kind="Internal")
    sup1a = nc.dram_tensor("sup1a", [cfg.n_a, H], dt.bfloat16,
                           kind="Internal", addr_space="Shared")
    sup1b = nc.dram_tensor("sup1b", [cfg.n_b, H], dt.bfloat16,
                           kind="Internal", addr_space="Shared")
    sup2_loc = nc.dram_tensor("sup2_loc", [cfg.rows_per_core, H], dt.bfloat16,
                              kind="Internal")
    sup2a = nc.dram_tensor("sup2a", [cfg.n_a, H], dt.bfloat16,
                           kind="Internal", addr_space="Shared")
    sup2b = nc.dram_tensor("sup2b", [cfg.n_b, H], dt.bfloat16,
                           kind="Internal", addr_space="Shared")

    groups = [list(range(N_CORES))]

    with tile.TileContext(nc) as tc:
        with (
            tc.tile_pool(name="meta", bufs=1) as meta,
            tc.tile_pool(name="sA", bufs=14) as sApool,
            tc.tile_pool(name="spool", bufs=6) as spool,
            tc.tile_pool(name="ps1", bufs=2, space="PSUM") as ps1p,
            tc.tile_pool(name="psA", bufs=2, space="PSUM") as psAp,
            tc.tile_pool(name="psB", bufs=4, space="PSUM") as psBp,
        ):
            lib = nc.gpsimd.load_library(library_config.mlp)

            # ---- constant / metadata loads ----
            idx_sb = meta.tile([P, T * 2 * ICOLS], dt.int16)
            nc.sync.dma_start(idx_sb[:], idx16[:])
            w1_sb = meta.tile([P, KC * H], dt.bfloat16)
            nc.sync.dma_start(w1_sb[:], w1[:])
            w2_sb = meta.tile([H, H], dt.bfloat16)
            nc.sync.dma_start(w2_sb[:], w2[:])
            b1_sb = meta.tile([H, 1], dt.float32)
            nc.sync.dma_start(b1_sb[:], b1[:])
            b2_sb = meta.tile([H, 1], dt.float32)
            nc.sync.dma_start(b2_sb[:], b2[:])
            dstv_sb = meta.tile([P, T * KB], dt.float32)
            nc.sync.dma_start(dstv_sb[:], dstv[:])
            valv_sb = meta.tile([P, T * KB], dt.float32)
            nc.sync.dma_start(valv_sb[:], valv[:])
            # iota row 0..127 along free dim, same in every partition;
            # generated as int32 then converted to bf16 (exact for 0..127)
            iota32 = meta.tile([P, P], dt.int32)
            nc.gpsimd.iota(iota32[:], pattern=[[1, P]], channel_multiplier=0)
            iota_bf = meta.tile([P, P], dt.bfloat16)
            nc.scalar.copy(iota_bf[:], iota32[:])

            # featT streamed in 4 slabs of tiles (pool freed after phase B)
            featB = tc.tile_pool(name="featB", bufs=1)
            featpool = featB.__enter__()
            featT_sb = featpool.tile([P, KC * cfg.rows_per_core], dt.bfloat16)
            _spmm_pools = None
            nslab = 4
            per = (T + nslab - 1) // nslab
            for sl in range(nslab):
                a = sl * per * P
                b = min(T, (sl + 1) * per) * P
                if a >= b:
                    continue
                for kc in range(KC):
                    off = kc * cfg.rows_per_core
                    nc.sync.dma_start(featT_sb[:, off + a:off + b],
                                      featT[:, off + a:off + b])

            def ag(ins_ap, out_t):
                nc.gpsimd.collective_compute(
                    "AllGather", mybir.AluOpType.bypass,
                    replica_groups=groups, ins=[ins_ap], outs=[out_t[:]])

            # ---- phase B: local support1 chunk; AG each half when ready --
            for t in range(T):
                ps = ps1p.tile([P, H], dt.float32, tag="ps1")
                for kc in range(KC):
                    nc.tensor.matmul(
                        ps[:],
                        lhsT=featT_sb[:, kc * cfg.rows_per_core + t * P:
                                      kc * cfg.rows_per_core + (t + 1) * P],
                        rhs=w1_sb[:, kc * H:(kc + 1) * H],
                        start=(kc == 0), stop=(kc == KC - 1))
                s1_sb = spool.tile([P, H], dt.bfloat16, tag="s1")
                nc.scalar.copy(s1_sb[:], ps[:])
                nc.sync.dma_start(sup1_loc[t * P:(t + 1) * P, :], s1_sb[:])
                if t == min(cfg.ta + 1, T - 1):
                    ag(sup1_loc[0:cfg.rows_a, :], sup1a)
            ag(sup1_loc[cfg.rows_a:, :], sup1b)
            featB.__exit__(None, None, None)
            _cms = [tc.tile_pool(name="gA", bufs=6),
                    tc.tile_pool(name="gB", bufs=6),
                    tc.tile_pool(name="mA", bufs=10),
                    tc.tile_pool(name="mB", bufs=10),
                    tc.tile_pool(name="mK", bufs=1)]
            gApool, gBpool, mApool, mBpool, mKpool = \
                [c.__enter__() for c in _cms]
            KEEP = 11   # tiles whose m stays in SBUF across both layers
            m_kept = {}

            def emit_gather(g, t, half, tab, ntiles):
                """one gather covering `ntiles` consecutive tiles' `half`
                blocks (idx16 is laid out group-blocked, see prep_inputs)"""
                k = ntiles * KH
                out_ap = g[:].rearrange("p (k h) -> p k h", k=k)
                c0 = (t * 2 + half * ntiles) * ICOLS
                gi = nc.gpsimd.dma_gather(
                    out_ap=out_ap, in_ap=tab[:],
                    idxs_ap=idx_sb[:, c0:c0 + ntiles * ICOLS],
                    num_idxs=ntiles * cfg.nidx,
                    num_idxs_reg=ntiles * cfg.nidx,
                    elem_size=H, single_packet=False,
                    queue_num=((t // GM) * 2 + half) % 4)
                add_dep_helper(lib.ins, gi.ins, sync=False,
                               reason="lib before gather")

            class SpmmPhase:
                """software-pipelined spmm: half-A work leads by LAG tiles.

                Per tile: gather half-A rows (pair-merged gathers), source
                the vals-scaled selection matrix (first NS chunks streamed
                from DRAM, the rest built on DVE from (dst, val) scalars),
                matmul-accumulate, stash the partial (PSUM -> SBUF).  LAG
                tiles later the half-B gather + chunks run (their table's
                AllGather has completed by then), the partials combine,
                relu(+bias), and out_cb consumes the transposed tile.
                """

                def __init__(self, tab_a, tab_b, bias_sb, out_cb, layer):
                    self.tab = (tab_a, tab_b)
                    self.bias_sb = bias_sb
                    self.out_cb = out_cb
                    self.layer = layer
                    self.sA = {}
                    self.pend = ({}, {})

                def _gather(self, t, half):
                    pool = (gApool, gBpool)[half]
                    pend = self.pend[half]
                    nt = min(GM, T - t)
                    g = pool.tile([P, nt * KH * P], dt.bfloat16,
                                  tag="gab"[half])
                    emit_gather(g, t, half, self.tab[half], nt)
                    for i in range(nt):
                        pend[t + i] = (g, i * KH * P)

                def _m_tile(self, t, half):
                    if t < KEEP:
                        # m is layer-independent: build once (layer 0) in a
                        # pinned buffer, reuse verbatim in layer 1
                        if self.layer == 1:
                            return m_kept[(t, half)]
                        m = mKpool.tile([P, KH * P], dt.bfloat16,
                                        tag=f"mk{t}_{half}")
                        m_kept[(t, half)] = m
                    else:
                        m = (mApool, mBpool)[half].tile(
                            [P, KH * P], dt.bfloat16, tag="mab"[half])
                    eng = (nc.sync, nc.scalar)[half]
                    eng.dma_start(m[:, :NS * P],
                                  m01[:, (t * 2 + half) * NS * P:
                                      (t * 2 + half + 1) * NS * P])
                    for j in range(NS, KH):
                        c = t * KB + half * KH + j
                        nc.vector.tensor_scalar(
                            out=m[:, j * P:(j + 1) * P], in0=iota_bf[:],
                            scalar1=dstv_sb[:, c:c + 1],
                            scalar2=valv_sb[:, c:c + 1],
                            op0=mybir.AluOpType.is_equal,
                            op1=mybir.AluOpType.mult)
                    return m

                def _chunks(self, t, half, ps):
                    g, off = self.pend[half].pop(t)
                    m = self._m_tile(t, half)
                    for j in range(KH):
                        nc.tensor.matmul(
                            ps[:], lhsT=g[:, off + j * P:off + (j + 1) * P],
                            rhs=m[:, j * P:(j + 1) * P],
                            start=(j == 0), stop=(j == KH - 1))

                def a_work(self, t):
                    if t % GM == 0:
                        self._gather(t, 0)
                    ps = psAp.tile([P, P], dt.float32, tag="cha")
                    self._chunks(t, 0, ps)
                    sa = sApool.tile([P, P], dt.float32, tag="sa")
                    nc.scalar.copy(sa[:], ps[:])
                    self.sA[t] = sa

                def b_work(self, t):
                    if t % GM == 0:
                        self._gather(t, 1)
                    ps = psBp.tile([P, P], dt.float32, tag="chb")
                    self._chunks(t, 1, ps)
                    # combine halves + relu(+bias)
                    sa = self.sA.pop(t)
                    hc = spool.tile([P, P], dt.float32, tag="hc")
                    nc.vector.tensor_tensor(out=hc[:], in0=ps[:], in1=sa[:],
                                            op=mybir.AluOpType.add)
                    hT = spool.tile([P, P],
                                    dt.bfloat16 if self.layer == 0
                                    else dt.float32, tag=f"hT{self.layer}")
                    nc.scalar.activation(hT[:], hc[:],
                                         mybir.ActivationFunctionType.Relu,
                                         bias=self.bias_sb[:], scale=1.0)
                    self.out_cb(t, hT)

            def l1_out(t, hT):
                ps2 = psBp.tile([P, H], dt.float32, tag="chb")
                nc.tensor.matmul(ps2[:], lhsT=hT[:], rhs=w2_sb[:],
                                 start=True, stop=True)
                s2_sb = spool.tile([P, H], dt.bfloat16, tag="s2")
                nc.scalar.copy(s2_sb[:], ps2[:])
                nc.sync.dma_start(sup2_loc[t * P:(t + 1) * P, :], s2_sb[:])

            def l2_out(t, oT):
                nc.sync.dma_start(outT[:, t * P:(t + 1) * P], oT[:])

            # ---- phases D & F: the two spmm layers, interleaved at the
            # boundary: phase F's half-A warmup runs during phase D's
            # half-B tail so the gather queues never drain.
            dph = SpmmPhase(sup1a, sup1b, b1_sb, l1_out, 0)
            fph = SpmmPhase(sup2a, sup2b, b2_sb, l2_out, 1)
            lag = min(LAG, T - 1)
            tmid = min(cfg.ta + lag + 2, T - 1)
            for t in range(T):
                dph.a_work(t)
                if t >= lag:
                    dph.b_work(t - lag)
                if t == tmid:
                    ag(sup2_loc[0:cfg.rows_a, :], sup2a)
            fh = 0
            for i, t in enumerate(range(T - lag, T)):
                dph.b_work(t)
                if i >= 2:
                    fph.a_work(fh)
                    fh += 1
            ag(sup2_loc[cfg.rows_a:, :], sup2b)
            for t in range(fh, T):
                fph.a_work(t)
                if t >= lag:
                    fph.b_work(t - lag)
            for t in range(T - lag, T):
                fph.b_work(t)
            for c in reversed(_cms):
                c.__exit__(None, None, None)

    nc.compile()
    return nc


def prep_inputs(features, adj_rows, adj_cols, adj_vals, W1, b1, W2, b2,
                cfg: Cfg):
    """Host-side sharding: per-core featT chunks and per-tile edge lists."""
    rows = np.asarray(adj_rows, dtype=np.int64)
    cols = np.asarray(adj_cols, dtype=np.int64)
    vals = np.asarray(adj_vals, dtype=np.float32)
    feats = np.asarray(features, dtype=np.float32)
    n, f_in = feats.shape
    T = cfg.tiles_per_core
    KH, KB, ICOLS, NIDX = cfg.kh, cfg.kb, cfg.icols, cfg.nidx
    n_tiles = T * N_CORES

    # pad features to n_pad rows
    featsp = np.zeros((cfg.n_pad, f_in), np.float32)
    featsp[:n] = feats

    # sort edges by destination tile, then split by source half
    # (half = tile-aligned half of the source's core chunk)
    tile_of = rows // P
    order = np.argsort(tile_of, kind="stable")
    rows_s, cols_s, vals_s = rows[order], cols[order], vals[order]
    src_core = cols_s // cfg.rows_per_core
    src_r = cols_s % cfg.rows_per_core
    half_s = (src_r >= cfg.rows_a).astype(np.int64)
    # position within the per-half core-major gather table
    pos_s = np.where(half_s == 0,
                     src_core * cfg.rows_a + src_r,
                     src_core * cfg.rows_b + (src_r - cfg.rows_a))
    counts = np.bincount(tile_of[order] * 2 + half_s, minlength=n_tiles * 2)
    kmax = counts.max()
    assert kmax <= NIDX, f"tile/half edge count {kmax} exceeds {NIDX}"

    # order edges by (tile, half) for slotting
    order2 = np.argsort(tile_of[order] * 2 + half_s, kind="stable")
    rows_s, cols_s, vals_s = rows_s[order2], cols_s[order2], vals_s[order2]
    pos_s = pos_s[order2]

    # slot positions within each (tile, half) group
    grp = np.repeat(np.arange(n_tiles * 2), counts)
    starts = np.concatenate([[0], np.cumsum(counts)[:-1]])
    slot = np.arange(len(rows_s)) - starts[grp]

    # dense [n_tiles, 2, NIDX] arrays, padded with idx=0 / val=0
    idx_d = np.zeros((n_tiles, 2, NIDX), np.int64)
    dst_d = np.zeros((n_tiles, 2, NIDX), np.int16)
    val_d = np.zeros((n_tiles, 2, NIDX), np.float32)
    tile_idx = grp // 2
    half_idx = grp % 2
    idx_d[tile_idx, half_idx, slot] = pos_s
    dst_d[tile_idx, half_idx, slot] = (rows_s % P).astype(np.int16)
    val_d[tile_idx, half_idx, slot] = vals_s

    import ml_dtypes
    bf16 = ml_dtypes.bfloat16

    # group-blocked gather order: per group of GM tiles starting at t0,
    # [a of t0..t0+s-1, b of t0..t0+s-1]
    GM = 2
    blk_t, blk_h = [], []
    t = 0
    while t < T:
        s = min(GM, T - t)
        blk_t += list(range(t, t + s)) * 2
        blk_h += [0] * s + [1] * s
        t += s
    blk_t = np.array(blk_t)
    blk_h = np.array(blk_h)

    in_maps = []
    for c in range(N_CORES):
        t0 = c * T
        chunk = featsp[c * cfg.rows_per_core:(c + 1) * cfg.rows_per_core]
        # [rows, f_in] -> [P, KC*rows]: chunk kc on cols [kc*rows, (kc+1)*rows)
        kc_n = cfg.f_in // P
        featT = np.ascontiguousarray(
            chunk.T.reshape(kc_n, P, cfg.rows_per_core).transpose(1, 0, 2)
            .reshape(P, kc_n * cfg.rows_per_core)).astype(bf16)
        # idx16: [P, T*2*ICOLS], pair-grouped blocks, 16-wrapped and
        # replicated over 8 groups
        idx_c = idx_d[t0:t0 + T][blk_t, blk_h]          # [T*2, NIDX]
        idx_c = idx_c.reshape(T * 2, ICOLS, 16).astype(np.int16)
        idx16 = np.tile(idx_c.transpose(0, 2, 1), (1, 8, 1))  # [T*2, 128, IC]
        idx16 = idx16.transpose(1, 0, 2).reshape(P, T * 2 * ICOLS)
        # vals-scaled one-hot selection matrices: the first NS chunks per
        # (tile, half) are streamed as dense bf16 (m01); all chunks' (dst,
        # val) scalars ship as fp32 for the on-chip DVE build of the rest
        NS = cfg.ns
        dst_c = dst_d[t0:t0 + T].reshape(T, KB, P)          # [t, jj, p]
        val_c = val_d[t0:t0 + T].reshape(T, KB, P)          # [t, jj, p]
        dstv = np.ascontiguousarray(
            dst_c.transpose(2, 0, 1).reshape(P, T * KB).astype(np.float32))
        valv = np.ascontiguousarray(
            val_c.transpose(2, 0, 1).reshape(P, T * KB).astype(np.float32))
        # streamed chunks: jj in [0, NS) and [KH, KH+NS) per tile
        sjj = np.concatenate([np.arange(NS), KH + np.arange(NS)])
        dst_s = dst_c[:, sjj]                               # [t, 2*NS, p]
        val_s = val_c[:, sjj]
        m01 = np.zeros((T, 2 * NS, P, P), bf16)             # [t, sj, p, d]
        ti, ji, pi = np.indices(dst_s.shape, sparse=True)
        m01[ti, ji, pi, dst_s] = val_s.astype(bf16)
        m01 = np.ascontiguousarray(
            m01.transpose(2, 0, 1, 3).reshape(P, T * 2 * NS * P))
        in_maps.append({
            "featT": featT,
            "w1": np.ascontiguousarray(
                np.asarray(W1, np.float32).reshape(kc_n, P, H)
                .transpose(1, 0, 2).reshape(P, kc_n * H)).astype(bf16),
            "w2": np.asarray(W2, np.float32).astype(bf16),
            "b1": np.asarray(b1, np.float32).reshape(H, 1),
            "b2": np.asarray(b2, np.float32).reshape(H, 1),
            "idx16": np.ascontiguousarray(idx16),
            "m01": m01,
            "dstv": dstv,
            "valv": valv,
        })
    return in_maps


_CACHED = {}


def run(features, adj_rows, adj_cols, adj_vals, W1, b1, W2, b2, cfg,
        trace=False):
    key = (cfg.n_pad, cfg.kh, cfg.tiles_per_core)
    if key not in _CACHED:
        _CACHED[key] = build_program(cfg)
    nc = _CACHED[key]
    in_maps = prep_inputs(features, adj_rows, adj_cols, adj_vals,
                          W1, b1, W2, b2, cfg)
    res = bass_utils.run_bass_kernel_spmd(nc, in_maps, list(range(N_CORES)),
                                          trace=trace)
    outs = [res.results[c]["outT"] for c in range(N_CORES)]
    full = np.concatenate([o.T for o in outs], axis=0)  # [n_pad, H]
    return full[:features.shape[0]].astype(np.float32), res


def kernel(features, adj_rows, adj_cols, adj_vals, W1, b1, W2, b2):
    rows = np.asarray(adj_rows, dtype=np.int64)
    cols = np.asarray(adj_cols, dtype=np.int64)
    # KH sized from the actual data (block-rounded max per (tile, half))
    cfg0 = Cfg(N_NODES, F_IN, H, tiles_per_core=49, kh=2)
    tile_of = rows // P
    half_s = ((cols % cfg0.rows_per_core) >= cfg0.rows_a).astype(np.int64)
    counts = np.bincount(tile_of * 2 + half_s,
                         minlength=cfg0.tiles_per_core * N_CORES * 2)
    kh = max(2, math.ceil(counts.max() / P))
    cfg = Cfg(N_NODES, F_IN, H, tiles_per_core=49, kh=kh)
    out, _ = run(features, adj_rows, adj_cols, adj_vals, W1, b1, W2, b2, cfg)
    return out

